# revision 8
# baseline (speedup 1.0000x reference)
"""Trainium2 Bass kernel for nn_Attention_73254962200646.

Reference computation (per batch element b, all shapes hardcoded):
  qkv = conv3x3(x, W_qkv, pad=1)            x:[8,512,32,32], W_qkv:[1536,512,3,3]
  q,k,v -> [g=8 heads, n=1024, d=64]
  attn  = (q @ k^T) / (|q| |k| + eps)       cosine-similarity attention
  out   = attn @ v -> [512,32,32]
  out   = conv1x1(out, W_out); BatchNorm2d (batch stats); ReLU

Distribution: data-parallel over batch B=8 across the 8 NeuronCores (one
image per core). All compute is core-local in bf16 (fp32 PSUM accumulation);
the only collective is a 4KB AllReduce of the BatchNorm partial sums.

Layouts on device (per core):
  x        : [ci_blk=4, 128, 34, 34] bf16, zero-padded spatial (host-prepped)
  conv q/k : normal orientation  -> [co 128, s 1024] via W-stationary matmuls
  conv v   : transposed orientation -> vT [s 128, cv 512] via x-window-stationary
             matmuls (so v lands with n on partitions, as the 2nd attn matmul needs)
  attnT    : [j 128, i 1024] per (head, jblk) = (k/kn)^T (q/qn), heads packed
             2-per-PE-array via partition halves (row/col tiling)
  norms    : 1/sqrt(sum q^2 + eps) via ones-matmul + sqrt + fast reciprocal,
             broadcast back over partitions with a tiny K=2 fp32 matmul
"""

import numpy as np
import ml_dtypes

import concourse.tile as tile
import concourse.mybir as mybir
from concourse import bacc, bass_utils

BF = ml_dtypes.bfloat16
SMOOTH = 1e-4
BN_EPS = 1e-5
NCORES = 8

_NC = None
LAST_RESULT = None


def _build():
    f32 = mybir.dt.float32
    bf = mybir.dt.bfloat16
    AF = mybir.ActivationFunctionType
    ALU = mybir.AluOpType

    nc = bacc.Bacc("TRN2", target_bir_lowering=False, debug=False,
                   num_devices=NCORES)
    xin = nc.dram_tensor("xpad", [4, 128, 34, 34], bf, kind="ExternalInput").ap()
    wqk = nc.dram_tensor("wqk", [12, 4, 128, 3, 3, 128], bf, kind="ExternalInput").ap()
    wo = nc.dram_tensor("wo", [4, 128, 512], bf, kind="ExternalInput").ap()
    ident = nc.dram_tensor("ident", [128, 128], bf, kind="ExternalInput").ap()
    gb = nc.dram_tensor("gb", [128, 8], f32, kind="ExternalInput").ap()
    ones2 = nc.dram_tensor("ones2", [128, 2], bf, kind="ExternalInput").ap()
    sel2 = nc.dram_tensor("sel2", [2, 128], f32, kind="ExternalInput").ap()
    out = nc.dram_tensor("out", [512, 1024], f32, kind="ExternalOutput").ap()

    with tile.TileContext(nc) as tc:
        with tc.tile_pool(name="sb", bufs=1) as sb, \
             tc.tile_pool(name="tp", bufs=2) as tp, \
             tc.tile_pool(name="ps", bufs=4, space="PSUM") as ps, \
             tc.tile_pool(name="dram", bufs=1, space="DRAM") as dram:

            xp = sb.tile([128, 4, 34, 34], bf, tag="xp")
            identt = sb.tile([128, 128], bf, tag="identt")
            wot = sb.tile([128, 4, 512], bf, tag="wot")
            gbt = sb.tile([128, 8], f32, tag="gbt")
            ones2t = sb.tile([128, 2], bf, tag="ones2t")
            sel2t = sb.tile([2, 128], f32, tag="sel2t")
            qhat = sb.tile([128, 4, 1024], bf, tag="qhat")
            khat = sb.tile([128, 4, 1024], bf, tag="khat")
            vT = sb.tile([128, 8, 512], bf, tag="vT")
            att = sb.tile([128, 4, 1024], bf, tag="att")
            yt = sb.tile([128, 4, 1024], f32, tag="yt")
            part = sb.tile([128, 16], f32, tag="part")
            ccs = sb.tile([128, 8], f32, tag="ccs")
            stats = sb.tile([128, 8], f32, tag="stats")
            epst = sb.tile([128, 1], f32, tag="epst")
            smt = sb.tile([2, 1], f32, tag="smt")

            for cb in range(4):
                nc.sync.dma_start(xp[:, cb], xin[cb])
                nc.sync.dma_start(wot[:, cb], wo[cb])
            nc.sync.dma_start(gbt[:], gb)
            nc.sync.dma_start(ones2t[:], ones2)
            nc.sync.dma_start(sel2t[:], sel2)
            nc.sync.dma_start(identt[:], ident)
            nc.vector.memset(epst[:], BN_EPS)
            nc.vector.memset(smt[:], SMOOTH)

            # ---- QKV conv (normal orientation). cob 8-11 are the v blocks:
            # those get PE-transposed to vT; q/k blocks get cosine norms.
            for cob in [8, 9, 10, 11, 0, 1, 2, 3, 4, 5, 6, 7]:
                is_v = cob >= 8
                is_q = cob < 4
                m = cob % 4
                wqt = tp.tile([128, 4, 3, 3, 128], bf, tag="wq", bufs=2,
                              name=f"wqt{cob}")
                for cb in range(4):
                    nc.sync.dma_start(wqt[:, cb], wqk[cob, cb])
                raw = tp.tile([128, 1024], bf, tag="raw", bufs=3,
                              name=f"raw{cob}")
                if not is_v:
                    nrm = tp.tile([2, 1024], f32, tag="nrm", bufs=2,
                                  name=f"nrm{cob}")
                    inv = tp.tile([2, 1024], f32, tag="inv", bufs=2,
                                  name=f"inv{cob}")
                for t in range(2):
                    pq = ps.tile([128, 512], f32, tag="mmp", bufs=4,
                                 name=f"pq{cob}_{t}")
                    k = 0
                    for cb in range(4):
                        for ky in range(3):
                            for kx in range(3):
                                nc.tensor.matmul(
                                    pq[:],
                                    wqt[:, cb, ky, kx, :],
                                    xp[:, cb, 16 * t + ky:16 * t + ky + 16, kx:kx + 32],
                                    start=(k == 0), stop=(k == 35))
                                k += 1
                    if t == 0:
                        nc.scalar.copy(raw[:, 0:512], pq[:])
                    else:
                        nc.vector.tensor_copy(out=raw[:, 512:1024], in_=pq[:])
                    if not is_v:
                        sq = tp.tile([128, 512], bf, tag="sq", bufs=2,
                                     name=f"sq{cob}_{t}")
                        nc.vector.tensor_mul(sq[:], raw[:, 512 * t:512 * (t + 1)],
                                             raw[:, 512 * t:512 * (t + 1)])
                        pss = ps.tile([2, 512], f32, tag="ss", bufs=2,
                                      name=f"pss{cob}_{t}")
                        nc.tensor.matmul(pss[:], ones2t[:], sq[:],
                                         start=True, stop=True)
                        nc.scalar.activation(out=nrm[:, 512 * t:512 * (t + 1)],
                                             in_=pss[:], func=AF.Sqrt,
                                             bias=smt[:], scale=1.0)
                if is_v:
                    # transpose [c,s] -> vT[s,c] in 128x128 chunks on the PE
                    for j in range(8):
                        pt = ps.tile([128, 128], bf, tag="mmp", bufs=4,
                                     name=f"pt{cob}_{j}")
                        nc.tensor.transpose(pt[:], raw[:, 128 * j:128 * (j + 1)],
                                            identt[:])
                        if j % 2 == 0:
                            nc.scalar.copy(vT[:, j, 128 * m:128 * (m + 1)], pt[:])
                        else:
                            nc.vector.tensor_copy(
                                out=vT[:, j, 128 * m:128 * (m + 1)], in_=pt[:])
                else:
                    nc.vector.reciprocal_approx_fast(out=inv[:], in_=nrm[:])
                    dst = qhat if is_q else khat
                    for t in range(2):
                        pbc = ps.tile([128, 512], f32, tag="mmp", bufs=4,
                                      name=f"pbc{cob}_{t}")
                        nc.tensor.matmul(pbc[:], sel2t[:],
                                         inv[:, 512 * t:512 * (t + 1)],
                                         start=True, stop=True)
                        nc.vector.tensor_mul(dst[:, m, 512 * t:512 * (t + 1)],
                                             raw[:, 512 * t:512 * (t + 1)], pbc[:])

            # ---- attention: per pair of heads (2m, 2m+1) ----
            for m in range(4):
                po = [ps.tile([128, 512], f32, tag="acc", bufs=2, name=f"po{m}_{t2}")
                      for t2 in range(2)]
                for j in range(8):
                    a0 = tp.tile([128, 1024], bf, tag="attnT", bufs=6)
                    a1 = tp.tile([128, 1024], bf, tag="attnT", bufs=6)
                    for t in range(2):
                        pa0 = ps.tile([128, 512], f32, tag="mmp", bufs=4)
                        nc.tensor.matmul(pa0[:],
                                         khat[0:64, m, 128 * j:128 * (j + 1)],
                                         qhat[0:64, m, 512 * t:512 * (t + 1)],
                                         start=True, stop=True)
                        pa1 = ps.tile([128, 512], f32, tag="mmp", bufs=4)
                        nc.tensor.matmul(pa1[:],
                                         khat[64:128, m, 128 * j:128 * (j + 1)],
                                         qhat[64:128, m, 512 * t:512 * (t + 1)],
                                         start=True, stop=True)
                        nc.scalar.copy(a0[:, 512 * t:512 * (t + 1)], pa0[:])
                        nc.vector.tensor_copy(out=a1[:, 512 * t:512 * (t + 1)], in_=pa1[:])
                    for t in range(2):
                        nc.tensor.matmul(po[t][0:64, :],
                                         vT[:, j, 128 * m:128 * m + 64],
                                         a0[:, 512 * t:512 * (t + 1)],
                                         start=(j == 0), stop=(j == 7),
                                         tile_position=(0, 0))
                        nc.tensor.matmul(po[t][64:128, :],
                                         vT[:, j, 128 * m + 64:128 * (m + 1)],
                                         a1[:, 512 * t:512 * (t + 1)],
                                         start=(j == 0), stop=(j == 7),
                                         tile_position=(0, 64))
                for t in range(2):
                    if t == 0:
                        nc.scalar.copy(att[:, m, 0:512], po[0][:])
                    else:
                        nc.vector.tensor_copy(out=att[:, m, 512:1024], in_=po[1][:])

            # ---- 1x1 conv + BN partial sums ----
            idx = 0
            for c4 in range(4):
                for t in range(2):
                    py = ps.tile([128, 512], f32, tag="mmp", bufs=4)
                    for cb in range(4):
                        nc.tensor.matmul(py[:],
                                         wot[:, cb, 128 * c4:128 * (c4 + 1)],
                                         att[:, cb, 512 * t:512 * (t + 1)],
                                         start=(cb == 0), stop=(cb == 3))
                    nc.vector.tensor_scalar(
                        out=yt[:, c4, 512 * t:512 * (t + 1)], in0=py[:],
                        scalar1=1.0, scalar2=None,
                        op0=ALU.mult, op1=ALU.add,
                        accum_out=part[:, idx:idx + 1])
                    bscr = tp.tile([128, 512], bf, tag="bscr", bufs=2)
                    nc.scalar.activation(out=bscr[:], in_=py[:], func=AF.Square,
                                         accum_out=part[:, 8 + idx:8 + idx + 1])
                    idx += 1

            # ---- BatchNorm: AllReduce 4KB of partial sums, then apply ----
            nc.vector.tensor_reduce(
                out=ccs[:], in_=part[:].rearrange("p (a b) -> p a b", b=2),
                axis=mybir.AxisListType.X, op=ALU.add)
            cin_d = dram.tile([128, 8], f32)
            cout_d = dram.tile([128, 8], f32)
            nc.gpsimd.dma_start(cin_d[:], ccs[:])
            nc.gpsimd.collective_compute(
                "AllReduce", ALU.add,
                ins=[cin_d[:].opt()], outs=[cout_d[:].opt()],
                replica_groups=[list(range(NCORES))])
            nc.sync.dma_start(stats[:], cout_d[:])

            mean = sb.tile([128, 4], f32, tag="mean")
            ex2 = sb.tile([128, 4], f32, tag="ex2")
            var = sb.tile([128, 4], f32, tag="var")
            stdt = sb.tile([128, 4], f32, tag="stdt")
            rstd = sb.tile([128, 4], f32, tag="rstd")
            scl = sb.tile([128, 4], f32, tag="scl")
            sht = sb.tile([128, 4], f32, tag="sht")
            msq = sb.tile([128, 4], f32, tag="msq")
            tmp = sb.tile([128, 4], f32, tag="tmp")
            NINV = 1.0 / 8192.0
            nc.vector.tensor_scalar_mul(mean[:], stats[:, 0:4], NINV)
            nc.vector.tensor_scalar_mul(ex2[:], stats[:, 4:8], NINV)
            nc.vector.tensor_mul(msq[:], mean[:], mean[:])
            nc.vector.tensor_sub(var[:], ex2[:], msq[:])
            nc.scalar.activation(out=stdt[:], in_=var[:], func=AF.Sqrt,
                                 bias=epst[:], scale=1.0)
            nc.vector.reciprocal_approx_fast(out=rstd[:], in_=stdt[:])
            nc.vector.tensor_mul(scl[:], gbt[:, 0:4], rstd[:])
            nc.vector.tensor_mul(tmp[:], mean[:], scl[:])
            nc.vector.tensor_sub(sht[:], gbt[:, 4:8], tmp[:])
            for c4 in range(4):
                nc.scalar.activation(out=yt[:, c4, :], in_=yt[:, c4, :],
                                     func=AF.Relu,
                                     scale=scl[:, c4:c4 + 1],
                                     bias=sht[:, c4:c4 + 1])
                nc.sync.dma_start(out[128 * c4:128 * (c4 + 1), :], yt[:, c4, :])

    nc.compile()
    return nc


def _prep_inputs(x, W_qkv, W_out, gamma, beta):
    x = np.asarray(x, np.float32)
    W_qkv = np.asarray(W_qkv, np.float32)
    W_out = np.asarray(W_out, np.float32)
    gamma = np.asarray(gamma, np.float32)
    beta = np.asarray(beta, np.float32)

    xs = x.reshape(8, 4, 128, 32, 32)
    xpad = np.zeros((8, 4, 128, 34, 34), np.float32)
    xpad[:, :, :, 1:33, 1:33] = xs
    xpad = xpad.astype(BF)

    wqk = np.ascontiguousarray(
        W_qkv.reshape(12, 128, 4, 128, 3, 3)
        .transpose(0, 2, 3, 4, 5, 1).astype(BF))
    wo = np.ascontiguousarray(
        W_out[:, :, 0, 0].T.reshape(4, 128, 512).astype(BF))
    gb = np.ascontiguousarray(np.concatenate(
        [gamma.reshape(4, 128).T, beta.reshape(4, 128).T], axis=1)
        .astype(np.float32))
    p = np.arange(128)
    ones2 = np.ascontiguousarray(
        np.stack([p < 64, p >= 64], axis=1).astype(BF))
    sel2 = np.ascontiguousarray(
        np.stack([p < 64, p >= 64], axis=0).astype(np.float32))

    identv = np.eye(128, dtype=BF)
    common = {"wqk": wqk, "wo": wo, "gb": gb,
              "ones2": ones2, "sel2": sel2, "ident": identv}
    return [{"xpad": np.ascontiguousarray(xpad[b]), **common}
            for b in range(8)]


def kernel(x, W_qkv, W_out, gamma, beta):
    global _NC, LAST_RESULT
    if _NC is None:
        _NC = _build()
    in_maps = _prep_inputs(x, W_qkv, W_out, gamma, beta)
    res = bass_utils.run_bass_kernel_spmd(
        _NC, in_maps, core_ids=list(range(NCORES)))
    LAST_RESULT = res
    outs = [res.results[b]["out"].reshape(512, 32, 32) for b in range(8)]
    return np.stack(outs).astype(np.float32)
